# revision 40
# baseline (speedup 1.0000x reference)
"""Causal self-attention (B=4, T=2048, C=768, H=12) on 8 trn2 NeuronCores.

Sharding: 8 cores = 4 batches x 2 head-groups (6 heads each).
Each core: QKV projection for its 6 heads, causal attention, partial output
projection (row-parallel). Host sums the two partials per batch + b_proj.

Device-side layout: fully transposed dataflow, bf16 matmul operands
(fp32 PSUM accumulation everywhere, bf16 partial outputs summed in fp32
on the host).
  - x shipped bf16 pre-transposed; Q^T/K^T [64, T] per head come from the
    QKV matmul (out = W.T @ x^T); V is computed in natural [T, 64] layout
    with a ones column appended (softmax denominator via the AV matmul).
  - Scores computed as S^T [k, q] (lhsT=K^T, rhs=Q^T), exp on ACT engine
    (1/sqrt(D) folded into activation scale), causal mask via mask-mul on
    DVE for diagonal blocks only.
  - Depth-1 software pipelining in the attention inner loop: the AV
    matmuls for k-block kb issue after the scores matmuls for kb+1, so
    the exp (ACT) for kb runs while the PE streams kb+1's scores instead
    of stalling the PE queue on the exp semaphore every block.
  - Startup DMAs spread across queues (sync: x, gpsimd+scalar: W_qk,
    scalar: the rest) so the first QKV matmul starts ASAP; the pending
    projection is braided between the last chunk's attention calls to
    cover the normalize-chain latency there.
"""

import os
import sys
import types

sys.path.insert(0, "/opt/trn_rl_repo")

import ml_dtypes
import numpy as np

import concourse.bass as bass
import concourse.tile as tile
from concourse import bacc, mybir
from concourse.bass_utils import run_bass_kernel_spmd

B, T, C, H, D = 4, 2048, 768, 12, 64
N_CORES = 8
HPC = H // 2          # heads per core = 6
FQK = 2 * HPC * D     # 768 qk features per core
FV = HPC * D          # 384 v features per core
E = D + 1             # 65: head dim + ones column
TT = T // 128         # 16 token tiles
CCH = C // 128        # 6 contraction chunks
QC = T // 512         # 4 query chunks of 512
F32 = mybir.dt.float32
BF16 = mybir.dt.bfloat16
NPBF = ml_dtypes.bfloat16


def _install_ntff_hook():
    """The image's antenv lacks axon_hooks; inject it so trace=True works."""
    if "antenv.axon_hooks" in sys.modules:
        return
    try:
        import antenv
        mod = types.ModuleType("antenv.axon_hooks")
        _state = {"hook": None}
        mod.set_axon_ntff_profile_hook = lambda h: _state.__setitem__("hook", h)
        mod.get_axon_ntff_profile_hook = lambda: _state["hook"]
        sys.modules["antenv.axon_hooks"] = mod
        antenv.axon_hooks = mod
        from trn_agent_boot.trn_boot import _ntff_profile_via_ctypes
        mod.set_axon_ntff_profile_hook(
            _ntff_profile_via_ctypes("/opt/axon/libaxon_pjrt.so")
        )
    except Exception:
        pass


def _build_program():
    nc = bacc.Bacc(
        "TRN2",
        target_bir_lowering=False,
        debug=False,
        enable_asserts=False,
        num_devices=N_CORES,
    )
    xtd = nc.dram_tensor("xtd", [C, T], BF16, kind="ExternalInput").ap()
    wqk = nc.dram_tensor("wqk", [C, FQK], BF16, kind="ExternalInput").ap()
    wv = nc.dram_tensor("wv", [C, HPC * E], BF16, kind="ExternalInput").ap()
    bqk = nc.dram_tensor("bqk", [FQK], F32, kind="ExternalInput").ap()
    bv = nc.dram_tensor("bv", [HPC * E], BF16, kind="ExternalInput").ap()
    wp = nc.dram_tensor("wp", [FV, C], BF16, kind="ExternalInput").ap()
    onesd = nc.dram_tensor("onesd", [128, 128], BF16, kind="ExternalInput").ap()
    maskd = nc.dram_tensor("maskd", [128, 512], BF16, kind="ExternalInput").ap()
    yp = nc.dram_tensor("yp", [T, C], BF16, kind="ExternalOutput").ap()

    with tile.TileContext(nc) as tc:
        _body(tc, nc, xtd, wqk, wv, bqk, bv, wp, onesd, maskd, yp)

    nc.compile()
    return nc


def _body(tc, nc, xtd, wqk, wv, bqk, bv, wp, onesd, maskd, yp):
    from contextlib import ExitStack

    with ExitStack() as es:
        persist = es.enter_context(tc.tile_pool(name="persist", bufs=1))
        # PSUM: mm512 x4 + pvpp x2 + yz x2 = 8 banks
        mm512 = es.enter_context(tc.tile_pool(name="mm512", bufs=4, space="PSUM"))
        pvpp = es.enter_context(tc.tile_pool(name="pvpp", bufs=2, space="PSUM"))
        psyz = es.enter_context(tc.tile_pool(name="psyz", bufs=2, space="PSUM"))
        zpool = es.enter_context(tc.tile_pool(name="zpool", bufs=6))
        ypool = es.enter_context(tc.tile_pool(name="ypool", bufs=2))
        opool = es.enter_context(tc.tile_pool(name="opool", bufs=3))
        spool = es.enter_context(tc.tile_pool(name="spool", bufs=4))

        # ---- persistent SBUF tiles
        ones_1x128 = persist.tile([1, 128], BF16, tag="ones128", name="ones_1x128")
        mask_sb = persist.tile([128, 512], BF16, tag="mask", name="mask_sb")
        bqk_sb = persist.tile([128, CCH], F32, tag="bqk", name="bqk_sb")
        bv_sb = persist.tile([1, HPC * E], BF16, tag="bv", name="bv_sb")
        # bv broadcast across partitions: adding it to the V' PSUM both applies
        # the bias and plants the ones column (wv's ones columns are zero, so
        # the matmul accumulates 0 there and the add supplies the 1.0).
        bv_bc = persist.tile([128, HPC * E], BF16, tag="bv_bc", name="bv_bc")
        wqk_sb = [persist.tile([128, FQK], BF16, tag=f"wqk{i}", name=f"wqk_sb{i}")
                  for i in range(CCH)]
        wv_sb = [persist.tile([128, HPC * E], BF16, tag=f"wv{i}", name=f"wv_sb{i}")
                 for i in range(CCH)]
        wp_sb = [persist.tile([128, C], BF16, tag=f"wp{i}", name=f"wp_sb{i}")
                 for i in range(FV // 128)]
        xT = [persist.tile([128, T], BF16, tag=f"xT{i}", name=f"xT{i}")
              for i in range(CCH)]
        # QK^T: tiles 0..2 hold Q^T (6 heads x 64), 3..5 hold K^T
        qkt = [persist.tile([128, T], BF16, tag=f"qkt{i}", name=f"qkt{i}")
               for i in range(CCH)]
        # V', one [128, 390] tile per token block: per head 64 V cols + ones col
        vp = [persist.tile([128, HPC * E], BF16, tag=f"vp{i}", name=f"vp{i}")
              for i in range(TT)]

        def a_chunk(t4):
            # DMA this chunk's x^T columns (pre-transposed on host): sync queue
            for cc in range(CCH):
                nc.sync.dma_start(
                    xT[cc][:, t4 * 512:(t4 + 1) * 512],
                    xtd[cc * 128:(cc + 1) * 128, t4 * 512:(t4 + 1) * 512],
                )

        # ---- startup: the first matmul needs xT[0]'s q4=0 slice and
        # wqk[0]. The sync queue exits the engine preamble last, so that
        # critical x slice goes first on scalar while wqk[0] goes first on
        # gpsimd — two early queues, no serialization between the pair.
        nc.scalar.dma_start(xT[0][:, 0:512], xtd[0:128, 0:512])
        for cc in range(1, CCH):
            nc.sync.dma_start(
                xT[cc][:, 0:512], xtd[cc * 128:(cc + 1) * 128, 0:512])
        for i in range(CCH):
            eng = nc.gpsimd if i % 2 == 0 else nc.scalar
            eng.dma_start(wqk_sb[i][:], wqk[i * 128:(i + 1) * 128, :])
        nc.scalar.dma_start(bqk_sb[:], bqk.rearrange("(f p) -> p f", p=128))
        nc.scalar.dma_start(ones_1x128[:], onesd[0:1, 0:128])
        nc.scalar.dma_start(mask_sb[:], maskd[:])
        nc.scalar.dma_start(bv_sb[:], bv[None, :])
        nc.gpsimd.partition_broadcast(bv_bc[:], bv_sb[:])
        for i in range(CCH):
            nc.scalar.dma_start(wv_sb[i][:], wv[i * 128:(i + 1) * 128, :])
        for i in range(FV // 128):
            nc.scalar.dma_start(wp_sb[i][:], wp[i * 128:(i + 1) * 128, :])

        def b_round(q4, r):
            # Q^T (ft=r) and K^T (ft=r+3) for head pair 2r, 2r+1
            group = [r, r + 3]
            tiles = [mm512.tile([128, 512], F32, tag="mm512", name=f"ps{i}")
                     for i in range(len(group))]
            for cc in range(CCH):
                for ft, ps in zip(group, tiles):
                    nc.tensor.matmul(
                        ps[:],
                        wqk_sb[cc][:, ft * 128:(ft + 1) * 128],
                        xT[cc][:, q4 * 512:(q4 + 1) * 512],
                        start=(cc == 0),
                        stop=(cc == CCH - 1),
                    )
            for ft, ps in zip(group, tiles):
                nc.vector.tensor_scalar_add(
                    qkt[ft][:, q4 * 512:(q4 + 1) * 512],
                    ps[:],
                    bqk_sb[:, ft:ft + 1],
                )

        def c_chunk(t4, js=(0, 1, 2, 3)):
            # V' tiles for this chunk's token blocks
            for j in js:
                tt = t4 * 4 + j
                pv = pvpp.tile([128, HPC * E], F32, tag="pvpp", name="pv")
                for cc in range(CCH):
                    nc.tensor.matmul(
                        pv[:],
                        xT[cc][:, tt * 128:(tt + 1) * 128],
                        wv_sb[cc][:],
                        start=(cc == 0),
                        stop=(cc == CCH - 1),
                    )
                nc.vector.tensor_add(vp[tt][:], pv[:], bv_bc[:])

        def kt_slice(h, kb):
            return qkt[3 + h // 2][(h % 2) * 64:(h % 2) * 64 + 64,
                                   kb * 128:(kb + 1) * 128]

        def q_slice(h, q4, off):
            return qkt[h // 2][(h % 2) * 64:(h % 2) * 64 + 64,
                               q4 * 512 + off:(q4 + 1) * 512]

        def attn_heads(q4, yts, heads):
            nkb = 4 * q4 + 4
            yzs = {h: psyz.tile([E, 512], F32, tag="yz", name=f"yz{h}")
                   for h in heads}
            # depth-1 software pipeline: the AV matmuls for block kb are
            # issued after the scores matmuls for block kb+1, so the exp
            # (ACT) for kb runs while the PE streams kb+1's scores.
            pending = None  # (kb, off, {h: zt})
            for kb in range(nkb):
                # diagonal blocks only need columns q >= kb*128
                off = max(0, kb * 128 - q4 * 512)
                w = 512 - off
                diag = kb * 128 >= q4 * 512
                zts = {}
                for h in heads:
                    sp = mm512.tile([128, 512], F32, tag="mm512", name="sp")
                    nc.tensor.matmul(
                        sp[:, off:512], kt_slice(h, kb),
                        q_slice(h, q4, off),
                        start=True, stop=True,
                    )
                    zt = zpool.tile([128, 512], BF16, tag="zt", name="zt")
                    nc.scalar.activation(
                        zt[:, off:512], sp[:, off:512],
                        mybir.ActivationFunctionType.Exp,
                        scale=1.0 / float(np.sqrt(D)),
                    )
                    if diag:  # causal mask: keep j' >= i after the offset trim
                        nc.vector.tensor_mul(
                            zt[:, off:512], zt[:, off:512], mask_sb[:, 0:w])
                    zts[h] = zt
                if pending is not None:
                    pkb, poff, pzts = pending
                    for h in heads:
                        nc.tensor.matmul(
                            yzs[h][:, poff:512],
                            vp[pkb][:, h * E:(h + 1) * E],
                            pzts[h][:, poff:512],
                            start=(pkb == 0), stop=False,
                        )
                pending = (kb, off, zts)
            pkb, poff, pzts = pending
            for h in heads:
                nc.tensor.matmul(
                    yzs[h][:, poff:512],
                    vp[pkb][:, h * E:(h + 1) * E],
                    pzts[h][:, poff:512],
                    start=(pkb == 0), stop=True,
                )
            for h in heads:
                yz = yzs[h]
                # normalize: y = yz[0:64] * (1/denom); recip on DVE,
                # broadcast on gpsimd.
                den0 = spool.tile([1, 512], F32, tag="den0", name="den0")
                nc.vector.tensor_copy(den0[:], yz[64:65, :])
                rc = spool.tile([1, 512], F32, tag="rc", name="rc")
                nc.vector.reciprocal_approx_fast(rc[:], den0[:])
                bc_sb = spool.tile([64, 512], F32, tag="bc_sb", name="bc_sb")
                nc.gpsimd.partition_broadcast(bc_sb[:], rc[:])
                nc.vector.tensor_mul(
                    yts[h // 2][(h % 2) * 64:(h % 2) * 64 + 64, :],
                    yz[0:64, :], bc_sb[:],
                )

        def proj_qt(q4, yts, qt):
            # bf16 partials (host sums the two cores' halves in fp32) and a
            # DMA per column-half right after its copy: the final row-block's
            # output transfer is what ends the kernel, so it should be small
            # and start as early as possible.
            ot = opool.tile([128, C], BF16, tag="ot", name="ot")
            row = (q4 * 4 + qt) * 128
            for half in range(2):
                pp = pvpp.tile([128, HPC * E], F32, tag="pvpp", name="pp")
                for hdc in range(FV // 128):
                    nc.tensor.matmul(
                        pp[:, 0:384],
                        yts[hdc][:, qt * 128:(qt + 1) * 128],
                        wp_sb[hdc][:, half * 384:(half + 1) * 384],
                        start=(hdc == 0), stop=(hdc == FV // 128 - 1),
                    )
                nc.vector.tensor_copy(
                    ot[:, half * 384:(half + 1) * 384], pp[:, 0:384])
                nc.sync.dma_start(
                    yp[row:row + 128, half * 384:(half + 1) * 384],
                    ot[:, half * 384:(half + 1) * 384])

        def proj_chunk(q4, yts):
            for qt in range(4):
                proj_qt(q4, yts, qt)

        # braided pipeline: next chunk's B-rounds interleave between this
        # chunk's head pairs
        for r in range(3):
            b_round(0, r)
        c_chunk(0)
        pending = None
        for q4 in range(QC):
            if q4 + 1 < QC:
                a_chunk(q4 + 1)
            yts = [ypool.tile([128, 512], BF16, tag=f"yt{i}", name=f"yt{i}")
                   for i in range(3)]
            last = q4 + 1 >= QC
            # braid the pending projection across this chunk's attention
            # calls: its PSUM->SBUF copies otherwise pile up in the in-order
            # DVE queue ahead of the diagonal mask muls
            if pending is not None:
                proj_qt(*pending, 0)
                proj_qt(*pending, 1)
            attn_heads(q4, yts, [0, 1])
            if pending is not None:
                proj_qt(*pending, 2)
            if not last:
                b_round(q4 + 1, 0)
            attn_heads(q4, yts, [2, 3])
            if pending is not None:
                proj_qt(*pending, 3)
            if not last:
                b_round(q4 + 1, 1)
            attn_heads(q4, yts, [4, 5])
            if not last:
                b_round(q4 + 1, 2)
                c_chunk(q4 + 1)
            pending = (q4, yts)
        proj_chunk(*pending)


_PROGRAM = None


def _get_program():
    global _PROGRAM
    if _PROGRAM is None:
        _PROGRAM = _build_program()
    return _PROGRAM


def _pad_wv(wv):
    out = np.zeros((C, HPC * E), dtype=NPBF)
    for h in range(HPC):
        out[:, h * E:h * E + D] = wv[:, h * D:(h + 1) * D].astype(NPBF)
    return out


def _pad_bv(bv):
    out = np.zeros((HPC * E,), dtype=NPBF)
    for h in range(HPC):
        out[h * E:h * E + D] = bv[h * D:(h + 1) * D].astype(NPBF)
        out[h * E + D] = 1.0
    return out


def kernel(x, W_attn, b_attn, W_proj, b_proj):
    x = np.ascontiguousarray(x, dtype=np.float32)
    W_attn = np.ascontiguousarray(W_attn, dtype=np.float32)
    b_attn = np.ascontiguousarray(b_attn, dtype=np.float32)
    W_proj = np.ascontiguousarray(W_proj, dtype=np.float32)
    b_proj = np.ascontiguousarray(b_proj, dtype=np.float32)

    nc = _get_program()
    ones_const = np.ones((128, 128), dtype=NPBF)
    mask_const = np.triu(np.ones((128, 512), np.float32)).astype(NPBF)

    in_maps = []
    for core in range(N_CORES):
        b, g = core // 2, core % 2
        qcols = slice(384 * g, 384 * (g + 1))
        kcols = slice(768 + 384 * g, 768 + 384 * (g + 1))
        vcols = slice(1536 + 384 * g, 1536 + 384 * (g + 1))
        in_maps.append({
            "xtd": np.ascontiguousarray(x[b].T).astype(NPBF),
            "wqk": np.concatenate(
                [W_attn[:, qcols], W_attn[:, kcols]], axis=1).astype(NPBF),
            "wv": _pad_wv(W_attn[:, vcols]),
            "bqk": np.ascontiguousarray(
                np.concatenate([b_attn[qcols], b_attn[kcols]])),
            "bv": _pad_bv(b_attn[vcols]),
            "wp": np.ascontiguousarray(
                W_proj[384 * g:384 * (g + 1), :]).astype(NPBF),
            "onesd": ones_const,
            "maskd": mask_const,
        })

    trace = bool(int(os.environ.get("KBENCH_TRACE", "0")))
    if trace:
        _install_ntff_hook()
    res = run_bass_kernel_spmd(
        nc, in_maps, list(range(N_CORES)), trace=trace,
    )
    kernel.last_exec_time_ns = res.exec_time_ns

    out = np.empty((B, T, C), dtype=np.float32)
    for b in range(B):
        out[b] = (res.results[2 * b]["yp"].astype(np.float32)
                  + res.results[2 * b + 1]["yp"].astype(np.float32)
                  + b_proj)
    return out
